# revision 1
# baseline (speedup 1.0000x reference)
"""Trainium2 Bass kernel: gated causal self-attention (GQA + partial RoPE).

Reference computation (per batch):
    q,k,v = x@Wq, x@Wk, x@Wv  (heads split, partial RoPE on first R dims)
    att = softmax(causal(q k^T / sqrt(D)))
    att = att * (att >= sigmoid(gate))          # post-softmax threshold gate
    y = (att @ v) @ Wo

Sharding over 8 NeuronCores: core = 4*b + g where b in {0,1} is the batch
(data parallel) and g in {0..3} is the KV-head group (tensor parallel:
Wq/Wk/Wv column-sharded, Wo row-sharded; gate sharded with heads).  Each
core computes a partial y^T (C x T); the host sums the 4 group partials
per batch and transposes.  The TxT score tensor never leaves a core.

On-chip layout: everything is computed transposed (qT/kT are (D,T),
scores are S^T = (s,t)) so that
  - softmax denominator = ones-matmul accumulation (and it lands
    partition-broadcast, exactly what the gate compare needs),
  - att@v needs no transposes: out^T accumulates with v-natural tiles as
    the stationary operand and gated exp(S^T) moving,
  - the output projection consumes out^T directly and emits y^T.

Precision split: the threshold-sensitive path (x, Wq, Wk, q^T, k^T, S^T)
runs float32r (FP22 multiply, FP32 accumulate, full PE rate); the
post-exp path (exp tiles, ones, v, Wo, out^T) runs float16, which turns
on Fast Weight Load for those matmuls and the DVE 2x mode for the
gating, at ~5e-4 relative cost on a purely linear/compare path.
exp() skips max-subtraction (scores are O(5), exp fits f16 range).
RoPE's rotate-half uses partition-shifted single-input copies plus
partition-aligned tensor_tensor ops; q-RoPE is batched across heads
with stride-0 broadcast APs for cos/sin.
"""

import numpy as np

import concourse.bass as bass
import concourse.tile as tile
from concourse import bacc, mybir
from concourse.alu_op_type import AluOpType
from concourse.bass_utils import run_bass_kernel_spmd

# Problem shapes (hardcoded per contract)
B, T, C = 2, 2048, 2048
H, HKV, D = 16, 4, 128
R = 64
NCORE = 8
G = 4            # tensor-parallel degree over KV heads
HL = H // G      # 4 local q heads per core
DL = HL * D      # 512 local q dims per core
SCALE = float(D) ** -0.5

F32 = mybir.dt.float32
F32R = mybir.dt.float32r
F16 = mybir.dt.float16
EXP = mybir.ActivationFunctionType.Exp

TB = 512                 # t-block width
NTB = T // TB            # 4
NCT = C // 128           # 16 contraction tiles
CQ = 4                   # c-tiles per xs chunk
NCHUNK = NCT // CQ       # 4 chunks
GB = 2                   # gating batch: s-tiles per DVE op

# packed f32 constant-tile column offsets: eye | thr
EYE0, THR0 = 0, 128
CONST_W = 128 + HL
# f16 mask tile: 4 diagonal masks (one per dpos) then a 128-wide ones block
ONES0 = 4 * TB
MSKS_W = 4 * TB + 128


def build():
    nc = bacc.Bacc("TRN2", target_bir_lowering=False, debug=False)

    xT = nc.dram_tensor("xT", [C, T], F32R, kind="ExternalInput").ap()
    wq = nc.dram_tensor("wq", [C, DL], F32R, kind="ExternalInput").ap()
    wk = nc.dram_tensor("wk", [C, D], F32R, kind="ExternalInput").ap()
    wv = nc.dram_tensor("wv", [C, D], F32R, kind="ExternalInput").ap()
    wo = nc.dram_tensor("wo", [DL, C], F16, kind="ExternalInput").ap()
    msks = nc.dram_tensor("msks", [128, MSKS_W], F16, kind="ExternalInput").ap()
    cs = nc.dram_tensor("cs", [R, T], F32, kind="ExternalInput").ap()
    sn = nc.dram_tensor("sn", [R, T], F32, kind="ExternalInput").ap()
    cst = nc.dram_tensor("cst", [128, CONST_W], F32, kind="ExternalInput").ap()
    ypT = nc.dram_tensor("ypT", [C, T], F32, kind="ExternalOutput").ap()

    with tile.TileContext(nc) as tc:
        with (
            tc.tile_pool(name="persist", bufs=1) as persist,
            tc.tile_pool(name="wpool", bufs=1) as wpool,
            tc.tile_pool(name="xpool", bufs=2) as xpool,
            tc.tile_pool(name="espool", bufs=2) as espool,
            tc.tile_pool(name="blk", bufs=2) as blk,
            tc.tile_pool(name="small", bufs=2) as small,
            tc.tile_pool(name="psum", bufs=1, space="PSUM") as psum,
        ):
            # ---- persistent SBUF ----
            kt = persist.tile([128, T], F32R)    # k^T (D x T), rope applied
            vn = persist.tile([128, T], F16)     # v natural; s-tile i at cols [128i,128i+128)
            cs_sb = persist.tile([R, T], F32)    # cos^T
            sn_sb = persist.tile([R, T], F32)    # sign-fixed sin^T: [-sinT[0:32] ; sinT[32:64]]
            msks_sb = persist.tile([128, MSKS_W], F16)
            cst_sb = persist.tile([128, CONST_W], F32)
            nc.sync.dma_start(cs_sb[:], cs)
            nc.sync.dma_start(sn_sb[:], sn)
            nc.sync.dma_start(msks_sb[:], msks)
            nc.sync.dma_start(cst_sb[:], cst)
            eye_sb = cst_sb[:, EYE0 : EYE0 + 128]
            thr_sb = cst_sb[:, THR0 : THR0 + HL]
            ones_sb = msks_sb[:, ONES0 : ONES0 + 128]

            # ---- weights (xs block 0 loads first, wo is deferred) ----
            wq_sb = wpool.tile([128, NCT, DL], F32R, tag="wq", name="wq_sb")
            wk_sb = wpool.tile([128, NCT, D], F32R, tag="wk", name="wk_sb")
            wv_sb = wpool.tile([128, NCT, D], F32R, tag="wv", name="wv_sb")
            wo_sb = wpool.tile([128, HL, C], F16, tag="wo", name="wo_sb")
            xs0_chunks = []
            for ch in range(NCHUNK):
                xs = xpool.tile([128, CQ, TB], F32R, tag="xs", name=f"xs_0_{ch}")
                for ci in range(CQ):
                    c = ch * CQ + ci
                    nc.sync.dma_start(xs[:, ci, :], xT[128 * c : 128 * (c + 1), 0:TB])
                xs0_chunks.append(xs)
            for c in range(NCT):
                csl = slice(128 * c, 128 * (c + 1))
                nc.sync.dma_start(wq_sb[:, c, :], wq[csl, :])
                nc.sync.dma_start(wk_sb[:, c, :], wk[csl, :])
                nc.sync.dma_start(wv_sb[:, c, :], wv[csl, :])

            def rope(th, dcols, tcols):
                """In-place partial RoPE on rows 0:R of region th[:, dcols].

                rotate-half via two partition-shifted single-input copies
                (legal on ACT), then partition-aligned tensor_tensor ops:
                  out[0:64] = q[0:64]*cos + rot*sin_signed
                with rot = [q[32:64]; q[0:32]], sin_signed = [-sin_lo; sin_hi].
                """
                hw = R // 2  # 32
                rot = small.tile([R, HL * TB], F32R, tag="ropeq", bufs=1, name="rope_rot")
                nc.scalar.copy(rot[0:hw, 0:TB], th[hw:R, dcols])
                nc.scalar.copy(rot[hw:R, 0:TB], th[0:hw, dcols])
                nc.vector.tensor_tensor(th[0:R, dcols], th[0:R, dcols], cs_sb[:, tcols], op=AluOpType.mult)
                nc.vector.tensor_tensor(rot[:, 0:TB], rot[:, 0:TB], sn_sb[:, tcols], op=AluOpType.mult)
                nc.vector.tensor_tensor(th[0:R, dcols], th[0:R, dcols], rot[:, 0:TB], op=AluOpType.add)

            def rope_q(qtb, tcols):
                """Batched RoPE over all HL head slices of qtb (same t-range),
                broadcasting cos/sin across the head dim with stride-0 APs."""
                hw = R // 2
                W = HL * TB
                rot = small.tile([R, W], F32R, tag="ropeq", bufs=1, name="ropeq_rot")
                nc.scalar.copy(rot[0:hw, :], qtb[hw:R, :])
                nc.scalar.copy(rot[hw:R, :], qtb[0:hw, :])
                qv = qtb[0:R, :].rearrange("p (r n) -> p r n", r=HL)
                rv = rot[:].rearrange("p (r n) -> p r n", r=HL)
                cb = cs_sb[:, tcols][:, None, :].broadcast_to([R, HL, TB])
                sb = sn_sb[:, tcols][:, None, :].broadcast_to([R, HL, TB])
                nc.vector.tensor_tensor(qv, qv, cb, op=AluOpType.mult)
                nc.vector.tensor_tensor(rv, rv, sb, op=AluOpType.mult)
                nc.vector.tensor_tensor(qv, qv, rv, op=AluOpType.add)

            # ---- main fully-unrolled t-block loop ----
            for j in range(NTB):
                tsl = slice(j * TB, (j + 1) * TB)

                # --- projections for block j ---
                if j == 0:
                    xs_chunks = xs0_chunks
                else:
                    xs_chunks = []
                    for ch in range(NCHUNK):
                        xs = xpool.tile([128, CQ, TB], F32R, tag="xs", name=f"xs_{j}_{ch}")
                        for ci in range(CQ):
                            c = ch * CQ + ci
                            nc.sync.dma_start(xs[:, ci, :], xT[128 * c : 128 * (c + 1), tsl])
                        xs_chunks.append(xs)

                # All 6 projection accumulators open at once; consume each
                # xs chunk fully before the next (xpool bufs=2 then suffices).
                qtb = blk.tile([128, HL * TB], F32R, tag="qtb", name=f"qtb_{j}")
                qps = [
                    psum.tile([128, TB], F32, tag="mm", bufs=4, name=f"qp_{j}_{h}")
                    for h in range(HL)
                ]
                kp = psum.tile([128, TB], F32, tag="acc", bufs=4, name=f"kp_{j}")
                vp = psum.tile([128, TB], F32, tag="acc", bufs=4, name=f"vp_{j}")
                groups = [(qps[h], wq_sb, 128 * h, 128) for h in range(HL)]
                groups += [(kp, wk_sb, 0, D), (vp, wv_sb, 0, D)]
                for ch in range(NCHUNK):
                    for gp, w_sb, col0, ncols in groups:
                        for ci in range(CQ):
                            c = ch * CQ + ci
                            nc.tensor.matmul(
                                gp[:],
                                w_sb[:, c, col0 : col0 + ncols],
                                xs_chunks[ch][:, ci, :],
                                start=(c == 0),
                                stop=(c == NCT - 1),
                            )
                for h in range(HL):
                    nc.scalar.copy(qtb[:, TB * h : TB * (h + 1)], qps[h][:])
                rope_q(qtb, tsl)
                nc.scalar.copy(kt[:, tsl], kp[:])
                rope(kt, tsl, tsl)
                vt_tmp = small.tile([128, TB], F32, tag="vt", bufs=1, name=f"vt_{j}")
                nc.scalar.copy(vt_tmp[:], vp[:])
                for u in range(TB // 128):
                    tp = psum.tile([128, 128], F32, tag="acc", bufs=4, name=f"tp_{j}_{u}")
                    nc.tensor.transpose(tp[:], vt_tmp[:, 128 * u : 128 * (u + 1)], eye_sb)
                    s_idx = j * (TB // 128) + u
                    nc.vector.tensor_copy(vn[:, 128 * s_idx : 128 * (s_idx + 1)], tp[:])

                if j == 0:
                    # wo is first needed by block 0's output projection; loading
                    # it here overlaps the DMA with block 0 compute instead of
                    # delaying the first matmul.
                    for d in range(HL):
                        nc.sync.dma_start(wo_sb[:, d, :], wo[128 * d : 128 * (d + 1), :])

                # --- attention for block j, all local heads ---
                nst = 4 * j + 4  # causal: s-tiles 0 .. 4j+3
                ytb = blk.tile([128, HL * TB], F16, tag="ytb", name=f"ytb_{j}")
                for h in range(HL):
                    qsl = slice(TB * h, TB * (h + 1))
                    esb = espool.tile([128, nst * TB], F16, tag="es", name=f"es_{j}_{h}")
                    # phase A: scores + exp (+ causal masks on the 4 diagonal tiles)
                    for i in range(nst):
                        ssl = slice(128 * i, 128 * (i + 1))
                        sp = psum.tile([128, TB], F32, tag="mm", bufs=4, name=f"sp_{j}_{h}_{i}")
                        nc.tensor.matmul(
                            sp[:], kt[:, ssl], qtb[:, qsl], start=True, stop=True
                        )
                        es = esb[:, TB * i : TB * (i + 1)]
                        nc.scalar.activation(es, sp[:], EXP, scale=SCALE)
                        dpos = i - 4 * j
                        if dpos >= 0:
                            # diagonal tile: mask dpos = [zeros(128*dpos) | tri | ones]
                            nc.vector.tensor_tensor(
                                es, es, msks_sb[:, TB * dpos : TB * (dpos + 1)],
                                op=AluOpType.mult,
                            )
                    # phase B: denominator (dense PE accumulation, f16+FWL)
                    dn = psum.tile([128, TB], F32, tag="acc", bufs=4, name=f"dn_{j}_{h}")
                    for i in range(nst):
                        nc.tensor.matmul(
                            dn[:], ones_sb, esb[:, TB * i : TB * (i + 1)],
                            start=(i == 0), stop=(i == nst - 1),
                        )
                    # phase C: threshold row (f16) and 1/denom (fast NR reciprocal)
                    work = small.tile([128, TB], F32, tag="work", bufs=2, name=f"work_{j}_{h}")
                    cwork = small.tile([128, TB], F16, tag="cwork", bufs=2, name=f"cwork_{j}_{h}")
                    cthr = cwork[:]
                    rden = work[:]
                    nc.vector.tensor_scalar_mul(cthr, dn[:], thr_sb[:, h : h + 1])
                    nc.vector.reciprocal_approx_fast(out=rden, in_=dn[:])
                    # phase D: batched gating, GB tiles per DVE op (f16, 2x mode)
                    for g0 in range(0, nst, GB):
                        gn = min(GB, nst - g0)
                        ev = esb[:, TB * g0 : TB * (g0 + gn)].rearrange(
                            "p (r n) -> p r n", r=gn
                        )
                        cb = cthr[:, None, :].broadcast_to([128, gn, TB])
                        msk = small.tile([128, GB * TB], F16, tag="msk", bufs=2, name=f"msk_{j}_{h}_{g0}")
                        mv = msk[:, 0 : TB * gn].rearrange("p (r n) -> p r n", r=gn)
                        nc.vector.tensor_tensor(mv, ev, cb, op=AluOpType.is_ge)
                        nc.vector.tensor_tensor(ev, ev, mv, op=AluOpType.mult)
                    # phase E: att @ v (dense, f16+FWL), then normalize
                    yp = psum.tile([128, TB], F32, tag="acc", bufs=4, name=f"yp_{j}_{h}")
                    for i in range(nst):
                        nc.tensor.matmul(
                            yp[:], vn[:, 128 * i : 128 * (i + 1)], esb[:, TB * i : TB * (i + 1)],
                            start=(i == 0), stop=(i == nst - 1),
                        )
                    nc.vector.tensor_tensor(ytb[:, qsl], yp[:], rden, op=AluOpType.mult)

                # --- output projection for block j (f16 + FWL) ---
                for co in range(C // 128):
                    op = psum.tile([128, TB], F32, tag="mm", bufs=4, name=f"op_{j}_{co}")
                    for d in range(HL):
                        nc.tensor.matmul(
                            op[:],
                            wo_sb[:, d, 128 * co : 128 * (co + 1)],
                            ytb[:, TB * d : TB * (d + 1)],
                            start=(d == 0),
                            stop=(d == HL - 1),
                        )
                    stg = small.tile([128, TB], F32, tag="stg", bufs=2, name=f"stg_{j}_{co}")
                    nc.scalar.copy(stg[:], op[:])
                    nc.sync.dma_start(ypT[128 * co : 128 * (co + 1), tsl], stg[:])

    nc.compile()
    return nc


_NC_CACHE = None


def _get_nc():
    global _NC_CACHE
    if _NC_CACHE is None:
        _NC_CACHE = build()
    return _NC_CACHE


def make_in_maps(x, cos, sin, Wq, Wk, Wv, Wo, gate):
    x = np.asarray(x, np.float32)
    cos = np.asarray(cos, np.float32)
    sin = np.asarray(sin, np.float32)
    Wq = np.asarray(Wq, np.float32)
    Wk = np.asarray(Wk, np.float32)
    Wv = np.asarray(Wv, np.float32)
    Wo = np.asarray(Wo, np.float32)
    gate = np.asarray(gate, np.float32)

    hw = R // 2
    cosT = np.ascontiguousarray(cos.T)  # (R, T)
    sinT = sin.T
    sn_signed = np.ascontiguousarray(np.concatenate([-sinT[0:hw], sinT[hw:R]], axis=0))
    thr_full = 1.0 / (1.0 + np.exp(-gate))  # sigmoid, (H,)
    tri = np.triu(np.ones((128, 128), np.float32))  # valid: s <= t
    cst_base = np.zeros((128, CONST_W), np.float32)
    cst_base[:, EYE0 : EYE0 + 128] = np.eye(128, dtype=np.float32)
    # f16 masks: for the diagonal s-tile at dpos, cols [0,128*dpos) invalid
    # (zeros), a 128-wide triangle at [128*dpos, ...), ones after.
    msks = np.zeros((128, MSKS_W), np.float16)
    for dpos in range(4):
        m = np.zeros((128, TB), np.float32)
        m[:, 128 * dpos : 128 * (dpos + 1)] = tri
        m[:, 128 * (dpos + 1) :] = 1.0
        msks[:, TB * dpos : TB * (dpos + 1)] = m
    msks[:, ONES0 : ONES0 + 128] = 1.0

    in_maps = []
    for core in range(NCORE):
        b, g = divmod(core, G)
        cst = cst_base.copy()
        cst[:, THR0 : THR0 + HL] = thr_full[HL * g : HL * (g + 1)]
        in_maps.append(
            {
                "xT": np.ascontiguousarray(x[b].T),
                "wq": np.ascontiguousarray(Wq[:, DL * g : DL * (g + 1)]),
                "wk": np.ascontiguousarray(Wk[:, D * g : D * (g + 1)]),
                "wv": np.ascontiguousarray(Wv[:, D * g : D * (g + 1)]),
                "wo": np.ascontiguousarray(Wo[DL * g : DL * (g + 1), :].astype(np.float16)),
                "msks": msks,
                "cs": cosT,
                "sn": sn_signed,
                "cst": cst,
            }
        )
    return in_maps


def run(inputs, trace=False, **kw):
    """Run on 8 NeuronCores; returns (y_full, BassKernelResults)."""
    nc = _get_nc()
    in_maps = make_in_maps(**inputs)
    res = run_bass_kernel_spmd(nc, in_maps, core_ids=list(range(NCORE)), trace=trace, **kw)
    y = np.zeros((B, T, C), np.float32)
    for core in range(NCORE):
        b = core // G
        y[b] += res.results[core]["ypT"].T
    return y, res


def kernel(**inputs) -> np.ndarray:
    y, _ = run(inputs)
    return y



# revision 2
# speedup vs baseline: 1.3345x; 1.3345x over previous
"""Trainium2 Bass kernel: gated causal self-attention (GQA + partial RoPE).

Reference computation (per batch):
    q,k,v = x@Wq, x@Wk, x@Wv  (heads split, partial RoPE on first R dims)
    att = softmax(causal(q k^T / sqrt(D)))
    att = att * (att >= sigmoid(gate))          # post-softmax threshold gate
    y = (att @ v) @ Wo

Sharding over 8 NeuronCores: core = 4*b + g where b in {0,1} is the batch
(data parallel) and g in {0..3} is the KV-head group (tensor parallel:
Wq/Wk/Wv column-sharded, Wo row-sharded; gate sharded with heads).  Each
core computes a partial y^T (C x T); the host sums the 4 group partials
per batch and transposes.  The TxT score tensor never leaves a core.

v2: the kernel is PE-stream-bound (242us of matmul columns at full
clock), so the emission order keeps the PE saturated:
  - h-major scores per t-block with den trailing one head and att@v two
    heads behind, so the ACT exp stream (2.7x slower than the score
    stream) never stalls the PE;
  - "filler" matmuls -- the NEXT block's projections (group-major over a
    single resident x block) and the PREVIOUS block's output projection
    -- are pumped between score/den/av groups;
  - gating is normalize-first: es' = es * (1/den), then one fused DVE
    pass (es' >= thr) * es' via scalar_tensor_tensor with a per-partition
    threshold scalar.  This kills the cthr build and the post-av
    normalize of the old flow;
  - causal masks are iota-predicated affine_select ops on the otherwise
    idle GpSimd engine; output-projection PSUM drains go to DVE; q/k run
    in f16 (scores band ~7e-4, comparable to the f16 es rounding).
"""

from collections import deque

import numpy as np

import concourse.bass as bass
import concourse.tile as tile
from concourse import bacc, mybir
from concourse.alu_op_type import AluOpType
from concourse.bass_utils import run_bass_kernel_spmd

B, T, C = 2, 2048, 2048
H, HKV, D = 16, 4, 128
R = 64
NCORE = 8
G = 4            # tensor-parallel degree over KV heads
HL = H // G      # 4 local q heads per core
DL = HL * D      # 512 local q dims per core
SCALE = float(D) ** -0.5

F32 = mybir.dt.float32
F32R = mybir.dt.float32r
F16 = mybir.dt.float16
EXP = mybir.ActivationFunctionType.Exp

TB = 512                 # t-block width
NTB = T // TB            # 4
NCT = C // 128           # 16 contraction tiles
GB = 2                   # s-tiles per batched DVE gating op

EYE0, THR0 = 0, 128
CONST_W = 128 + HL


def build():
    nc = bacc.Bacc("TRN2", target_bir_lowering=False, debug=False)

    xT = nc.dram_tensor("xT", [C, T], F32R, kind="ExternalInput").ap()
    wq = nc.dram_tensor("wq", [C, DL], F32R, kind="ExternalInput").ap()
    wk = nc.dram_tensor("wk", [C, D], F32R, kind="ExternalInput").ap()
    wv = nc.dram_tensor("wv", [C, D], F32R, kind="ExternalInput").ap()
    wo = nc.dram_tensor("wo", [DL, C], F16, kind="ExternalInput").ap()
    ones = nc.dram_tensor("ones", [128, 128], F16, kind="ExternalInput").ap()
    cs = nc.dram_tensor("cs", [R, T], F16, kind="ExternalInput").ap()
    sn = nc.dram_tensor("sn", [R, T], F16, kind="ExternalInput").ap()
    cst = nc.dram_tensor("cst", [128, CONST_W], F32, kind="ExternalInput").ap()
    ypT = nc.dram_tensor("ypT", [C, T], F16, kind="ExternalOutput").ap()

    with tile.TileContext(nc) as tc:
        with (
            tc.tile_pool(name="persist", bufs=1) as persist,
            tc.tile_pool(name="wpool", bufs=1) as wpool,
            tc.tile_pool(name="xpool", bufs=1) as xpool,
            tc.tile_pool(name="espool", bufs=1) as espool,
            tc.tile_pool(name="qpool", bufs=2) as qpool,
            tc.tile_pool(name="ypool", bufs=1) as ypool,
            tc.tile_pool(name="small", bufs=1) as small,
            tc.tile_pool(name="psum", bufs=1, space="PSUM") as psum,
        ):
            # ---- persistent SBUF ----
            kt = persist.tile([128, T], F16)     # k^T (D x T), rope applied
            vn = persist.tile([128, T], F16)     # v natural; s-tile i at cols 128i
            cs_sb = persist.tile([R, T], F16)    # cos^T
            sn_sb = persist.tile([R, T], F16)    # [-sinT[0:32] ; sinT[32:64]]
            ones_sb = persist.tile([128, 128], F16)
            cst_sb = persist.tile([128, CONST_W], F32)
            nc.sync.dma_start(cs_sb[:], cs)
            nc.sync.dma_start(sn_sb[:], sn)
            nc.sync.dma_start(ones_sb[:], ones)
            nc.sync.dma_start(cst_sb[:], cst)
            eye_sb = cst_sb[:, EYE0 : EYE0 + 128]

            wq_sb = wpool.tile([128, NCT, DL], F32R, tag="wq", name="wq_sb")
            wk_sb = wpool.tile([128, NCT, D], F32R, tag="wk", name="wk_sb")
            wv_sb = wpool.tile([128, NCT, D], F32R, tag="wv", name="wv_sb")
            wo_sb = wpool.tile([128, HL, C], F16, tag="wo", name="wo_sb")
            xs = xpool.tile([128, NCT, TB], F32R, tag="xs", name="xs")
            for c in range(NCT):
                csl = slice(128 * c, 128 * (c + 1))
                nc.sync.dma_start(xs[:, c, :], xT[csl, 0:TB])
                nc.sync.dma_start(wq_sb[:, c, :], wq[csl, :])
                nc.sync.dma_start(wk_sb[:, c, :], wk[csl, :])
                nc.sync.dma_start(wv_sb[:, c, :], wv[csl, :])
            for d in range(HL):
                nc.sync.dma_start(wo_sb[:, d, :], wo[128 * d : 128 * (d + 1), :])

            es = [
                espool.tile([128, NTB * 4 * TB], F16, tag=f"es{h}", name=f"es{h}")
                for h in range(HL)
            ]
            qtbs = {}
            ytb = ypool.tile([128, HL * TB], F16, tag="ytb", name="ytb")

            # ---- PE filler machinery ----
            fillers = deque()  # (kind, closure) ; closure emits ~4 matmuls

            def pump(n=1):
                for _ in range(n):
                    if fillers:
                        fillers.popleft()[1]()

            def flush(kind=None):
                while fillers and (kind is None or fillers[0][0] == kind):
                    fillers.popleft()[1]()

            def rope(th, dcols, tcols, name):
                """partial RoPE in place on rows 0:R of th[:, dcols] (f16)."""
                hw = R // 2
                rot = small.tile([R, TB], F16, tag="rot", bufs=1, name=f"rot_{name}")
                nc.scalar.copy(rot[0:hw, :], th[hw:R, dcols])
                nc.scalar.copy(rot[hw:R, :], th[0:hw, dcols])
                nc.vector.tensor_tensor(
                    th[0:R, dcols], th[0:R, dcols], cs_sb[:, tcols], op=AluOpType.mult
                )
                nc.vector.tensor_tensor(
                    rot[:], rot[:], sn_sb[:, tcols], op=AluOpType.mult
                )
                nc.vector.tensor_tensor(
                    th[0:R, dcols], th[0:R, dcols], rot[:], op=AluOpType.add
                )

            def make_proj_units(pj):
                """Filler units computing q/k/v for block pj from the resident
                xs (group-major: one PSUM accumulator live at a time)."""
                tsl = slice(pj * TB, (pj + 1) * TB)
                qtb = qpool.tile([128, HL * TB], F16, tag="qtb", name=f"qtb_{pj}")
                qtbs[pj] = qtb
                units = []

                def group(w_sb, col0, ncols, drain):
                    gp = psum.tile([128, TB], F32, tag="prj", bufs=2, name=f"prj_{pj}_{col0}_{ncols}")
                    for cu in range(4):
                        def u(gp=gp, cu=cu, w_sb=w_sb, col0=col0, ncols=ncols, drain=drain):
                            for c in range(4 * cu, 4 * cu + 4):
                                nc.tensor.matmul(
                                    gp[:],
                                    w_sb[:, c, col0 : col0 + ncols],
                                    xs[:, c, :],
                                    start=(c == 0),
                                    stop=(c == NCT - 1),
                                )
                            if cu == 3:
                                drain(gp)
                        units.append(("proj", u))

                for h in range(HL):
                    def qdrain(gp, h=h):
                        dsl = slice(TB * h, TB * (h + 1))
                        nc.scalar.copy(qtb[:, dsl], gp[:])
                        rope(qtb, dsl, tsl, f"q{pj}_{h}")
                    group(wq_sb, 128 * h, 128, qdrain)

                def kdrain(gp):
                    nc.scalar.copy(kt[:, tsl], gp[:])
                    rope(kt, tsl, tsl, f"k{pj}")
                group(wk_sb, 0, D, kdrain)

                vt = small.tile([128, TB], F32, tag="vt", bufs=1, name=f"vt_{pj}")

                def vdrain(gp):
                    nc.scalar.copy(vt[:], gp[:])
                group(wv_sb, 0, D, vdrain)

                def vtrans():
                    for u in range(TB // 128):
                        tp = psum.tile([128, 128], F32, tag="tp", bufs=1, name=f"tp_{pj}_{u}")
                        nc.tensor.transpose(tp[:], vt[:, 128 * u : 128 * (u + 1)], eye_sb)
                        s_idx = pj * 4 + u
                        nc.vector.tensor_copy(vn[:, 128 * s_idx : 128 * (s_idx + 1)], tp[:])
                units.append(("proj", vtrans))
                return units

            def make_outproj_units(oj):
                """Filler units for block oj's output projection (reads ytb,
                which is single-buffered: all units must run before block
                oj+1's first ytb write)."""
                tsl = slice(oj * TB, (oj + 1) * TB)
                units = []
                for co in range(NCT):
                    def u(co=co):
                        op = psum.tile([128, TB], F32, tag="acc", bufs=2, name=f"op_{oj}_{co}")
                        for d in range(HL):
                            nc.tensor.matmul(
                                op[:],
                                wo_sb[:, d, 128 * co : 128 * (co + 1)],
                                ytb[:, TB * d : TB * (d + 1)],
                                start=(d == 0),
                                stop=(d == HL - 1),
                            )
                        stg = small.tile([128, TB], F16, tag="stg", bufs=2, name=f"stg_{oj}_{co}")
                        nc.vector.tensor_copy(stg[:], op[:])
                        nc.sync.dma_start(ypT[128 * co : 128 * (co + 1), tsl], stg[:])
                    units.append(("out", u))
                return units

            # ---- block 0 projections (preamble, nothing to overlap yet) ----
            for _, u in make_proj_units(0):
                u()

            for j in range(NTB):
                tsl = slice(j * TB, (j + 1) * TB)
                nst = 4 * j + 4
                qtb = qtbs[j]

                # prefetch next x block over the just-freed xs buffer
                if j < NTB - 1:
                    for c in range(NCT):
                        nc.sync.dma_start(
                            xs[:, c, :], xT[128 * c : 128 * (c + 1), tsl.stop : tsl.stop + TB]
                        )

                # fillers: previous block's outproj first (ytb single-buffer),
                # then next block's projections
                if j > 0:
                    fillers.extend(make_outproj_units(j - 1))
                if j < NTB - 1:
                    fillers.extend(make_proj_units(j + 1))

                def emit_scores(h):
                    qsl = slice(TB * h, TB * (h + 1))
                    for i in range(nst):
                        sp = psum.tile([128, TB], F32, tag="sp", bufs=2, name=f"sp_{j}_{h}_{i}")
                        nc.tensor.matmul(
                            sp[:], kt[:, 128 * i : 128 * (i + 1)], qtb[:, qsl],
                            start=True, stop=True,
                        )
                        est = es[h][:, TB * i : TB * (i + 1)]
                        nc.scalar.activation(est, sp[:], EXP, scale=SCALE)
                        dpos = i - 4 * j
                        if dpos >= 0:
                            # causal: keep where t_glob >= s_glob, i.e.
                            # (512j - 128i) - p + f >= 0
                            nc.gpsimd.affine_select(
                                out=est,
                                in_=est,
                                pattern=[[1, TB]],
                                compare_op=AluOpType.is_ge,
                                fill=0.0,
                                base=512 * j - 128 * i,
                                channel_multiplier=-1,
                            )
                        if i % 2 == 1:
                            pump(1)

                def emit_den(h):
                    dn = psum.tile([128, TB], F32, tag="dn", bufs=1, name=f"dn_{j}_{h}")
                    for i in range(nst):
                        nc.tensor.matmul(
                            dn[:], ones_sb, es[h][:, TB * i : TB * (i + 1)],
                            start=(i == 0), stop=(i == nst - 1),
                        )
                        if i % 4 == 3:
                            pump(1)
                    rden32 = small.tile([128, TB], F32, tag="rden32", bufs=1, name=f"rd32_{j}_{h}")
                    rden16 = small.tile([128, TB], F16, tag="rden16", bufs=1, name=f"rd16_{j}_{h}")
                    nc.vector.reciprocal_approx_fast(out=rden32[:], in_=dn[:])
                    nc.vector.tensor_copy(rden16[:], rden32[:])
                    thr_ap = cst_sb[:, THR0 + h : THR0 + h + 1]
                    for g0 in range(0, nst, GB):
                        gn = min(GB, nst - g0)
                        ev = es[h][:, TB * g0 : TB * (g0 + gn)].rearrange(
                            "p (r n) -> p r n", r=gn
                        )
                        rb = rden16[:][:, None, :].broadcast_to([128, gn, TB])
                        nc.vector.tensor_tensor(ev, ev, rb, op=AluOpType.mult)
                        nc.vector.scalar_tensor_tensor(
                            ev, ev, thr_ap, ev, op0=AluOpType.is_ge, op1=AluOpType.mult
                        )

                def emit_av(h):
                    yp = psum.tile([128, TB], F32, tag="acc", bufs=2, name=f"yp_{j}_{h}")
                    for i in range(nst):
                        nc.tensor.matmul(
                            yp[:], vn[:, 128 * i : 128 * (i + 1)],
                            es[h][:, TB * i : TB * (i + 1)],
                            start=(i == 0), stop=(i == nst - 1),
                        )
                        if i % 4 == 3:
                            pump(1)
                    nc.scalar.copy(ytb[:, TB * h : TB * (h + 1)], yp[:])

                for h in range(HL):
                    emit_scores(h)
                    if h >= 1:
                        emit_den(h - 1)
                    if h == 2:
                        # ytb single-buffer: previous block's outproj must be
                        # fully emitted before this block's first ytb write
                        flush("out")
                    if h >= 2:
                        emit_av(h - 2)
                emit_den(HL - 1)
                emit_av(HL - 2)
                emit_av(HL - 1)
                flush()

            # tail: last block's output projection
            for _, u in make_outproj_units(NTB - 1):
                u()

    nc.compile()
    return nc


_NC_CACHE = None


def _get_nc():
    global _NC_CACHE
    if _NC_CACHE is None:
        _NC_CACHE = build()
    return _NC_CACHE


def make_in_maps(x, cos, sin, Wq, Wk, Wv, Wo, gate):
    x = np.asarray(x, np.float32)
    cos = np.asarray(cos, np.float32)
    sin = np.asarray(sin, np.float32)
    Wq = np.asarray(Wq, np.float32)
    Wk = np.asarray(Wk, np.float32)
    Wv = np.asarray(Wv, np.float32)
    Wo = np.asarray(Wo, np.float32)
    gate = np.asarray(gate, np.float32)

    hw = R // 2
    cosT = np.ascontiguousarray(cos.T).astype(np.float16)  # (R, T)
    sinT = sin.T
    sn_signed = np.ascontiguousarray(
        np.concatenate([-sinT[0:hw], sinT[hw:R]], axis=0)
    ).astype(np.float16)
    thr_full = 1.0 / (1.0 + np.exp(-gate))  # sigmoid, (H,)
    cst_base = np.zeros((128, CONST_W), np.float32)
    cst_base[:, EYE0 : EYE0 + 128] = np.eye(128, dtype=np.float32)
    ones16 = np.ones((128, 128), np.float16)

    in_maps = []
    for core in range(NCORE):
        b, g = divmod(core, G)
        cst = cst_base.copy()
        cst[:, THR0 : THR0 + HL] = thr_full[HL * g : HL * (g + 1)]
        in_maps.append(
            {
                "xT": np.ascontiguousarray(x[b].T),
                "wq": np.ascontiguousarray(Wq[:, DL * g : DL * (g + 1)]),
                "wk": np.ascontiguousarray(Wk[:, D * g : D * (g + 1)]),
                "wv": np.ascontiguousarray(Wv[:, D * g : D * (g + 1)]),
                "wo": np.ascontiguousarray(Wo[DL * g : DL * (g + 1), :].astype(np.float16)),
                "ones": ones16,
                "cs": cosT,
                "sn": sn_signed,
                "cst": cst,
            }
        )
    return in_maps


def run(inputs, trace=False, **kw):
    """Run on 8 NeuronCores; returns (y_full, BassKernelResults)."""
    nc = _get_nc()
    in_maps = make_in_maps(**inputs)
    res = run_bass_kernel_spmd(nc, in_maps, core_ids=list(range(NCORE)), trace=trace, **kw)
    y = np.zeros((B, T, C), np.float32)
    for core in range(NCORE):
        b = core // G
        y[b] += res.results[core]["ypT"].T.astype(np.float32)
    return y, res


def kernel(**inputs) -> np.ndarray:
    y, _ = run(inputs)
    return y
